# revision 7
# baseline (speedup 1.0000x reference)
"""Trainium2 Bass kernel for nn_AttachmentPredictor.

Data-parallel over batch across 8 cores (32 batches/core). Key ideas:

1. Mask compaction: the reference zeroes masked positions before the
   exp-normalization, so only unmasked head positions (~127 of 254 per
   batch) need the projection/hidden/scorer pipeline at all. The host
   gathers each batch's unmasked columns into a fixed-width stripe of
   Mb columns (global max count rounded up to 16), so the device sees
   N = 32*Mb columns per core instead of 32*256.
2. fp8 matmuls in DoubleRow perf mode (two 128-row contraction halves
   per instruction at 0.5 cycles/row) for stages 1-3, with optional
   hi/lo error compensation: hi parts in e4m3, lo parts in e5m2 (whose
   normal range covers the small lo magnitudes; e4m3's subnormal floor
   would swallow them).
3. The per-batch bias (prep+child projections) is folded into stage 1
   as 32 augmented one-hot contraction rows in f32r, so tanh needs no
   per-batch segmentation and the bias path stays near-exact.
4. Software-pipelined emission: stage 1 of block g+1 is issued to the
   in-order PE queue before stages 2/3 of block g, so the PE never
   waits on a same-block tanh and stays at full p-state.
5. Scorer in bf16 into a [1, 512] psum per block; scores are reshaped
   to [32, Mb] via a DRAM bounce and exp-normalized per batch on 32
   partitions; the host scatters rows back into the [256, 254] zeros.
"""

import ml_dtypes
import numpy as np

import concourse.bass as bass
import concourse.mybir as mybir
import concourse.tile as tile
from concourse import bass_utils

F32 = mybir.dt.float32
F32R = mybir.dt.float32r
BF16 = mybir.dt.bfloat16
F8 = mybir.dt.float8e4
F8L = mybir.dt.float8e5
AF = mybir.ActivationFunctionType
DR = mybir.MatmulPerfMode.DoubleRow

B, S, D, P = 256, 256, 1024, 512
NCORES = 8
BC = B // NCORES            # 32 batches per core
KD = 4                      # DoubleRow k-tiles over D (256 rows each)
KP = 2                      # DoubleRow k-tiles over P
EPS = 1e-7
NEG = -1e9

NP_F8 = ml_dtypes.float8_e4m3
NP_F8L = ml_dtypes.float8_e5m2
NP_BF16 = ml_dtypes.bfloat16

OPTS = {
    "c1_hilo": True,      # hi/lo split of stage-2 moving operand
    "c2_hilo": True,      # hi/lo split of stage-3 moving operand
    "w_hilo": False,      # hi/lo split of W0/W1 (lo x hi cross terms)
    "wh_hilo": False,     # hi/lo split of Wh
    "x_hilo": False,      # hi/lo split of x
    "xr_bufs": 4,
    "c_bufs": 6,
    "ps_bufs": 7,
}


# ---------------------------------------------------------------------------
# walrus in this container accepts at most ONE sync wait per instruction;
# split extra waits onto preceding NoOps on the same engine.
def _split_waits(nc, maxw=1):
    ctr = 0
    for f in nc.m.functions:
        for blk in f.blocks:
            insts = blk.instructions
            newlist = []
            changed = False
            for inst in insts:
                si = inst.sync_info
                if si is not None and len(si.on_wait) > maxw:
                    waits = list(si.on_wait)
                    keep = waits[len(waits) - maxw:]
                    extra = waits[: len(waits) - maxw]
                    for j in range(0, len(extra), maxw):
                        ctr += 1
                        newlist.append(
                            mybir.InstNoOp(
                                name=f"waitsplit-{ctr}",
                                engine=inst.engine,
                                ins=[],
                                outs=[],
                                sync_info=mybir.SyncInfo(
                                    on_wait=extra[j: j + maxw], on_update=[]
                                ),
                            )
                        )
                    inst.sync_info = mybir.SyncInfo(
                        on_wait=keep, on_update=list(si.on_update)
                    )
                    changed = True
                newlist.append(inst)
            if changed:
                insts[:] = newlist


def _dr3(ap2d, h=2):
    """[p, 2*n] AP -> [p, 2, n] DoubleRow operand view."""
    return ap2d.rearrange("p (h n) -> p h n", h=h)


# ---------------------------------------------------------------------------
def _build(nblk, opts=None, reps=1):
    opts = dict(OPTS, **(opts or {}))
    nc = bass.Bass("TRN2", target_bir_lowering=False, debug=False)

    N = nblk * 512              # compacted columns per core
    Mb = N // BC                # columns per batch stripe
    c1h, c2h = opts["c1_hilo"], opts["c2_hilo"]
    wh_, whh, xh = opts["w_hilo"], opts["wh_hilo"], opts["x_hilo"]

    xT_d = nc.dram_tensor("xT", [nblk, 128, KD * 1024], F8,
                          kind="ExternalInput").ap()
    xTl_d = nc.dram_tensor("xTl", [nblk, 128, KD * 1024], F8L,
                           kind="ExternalInput").ap()
    oh_d = nc.dram_tensor("oh", [BC, nblk * 512], F32R,
                          kind="ExternalInput").ap()
    wh_d = nc.dram_tensor("wh", [128, KD * 1024], F8,
                          kind="ExternalInput").ap()
    whl_d = nc.dram_tensor("whl", [128, KD * 1024], F8L,
                           kind="ExternalInput").ap()
    w0_d = nc.dram_tensor("w0", [128, KP * 1024], F8,
                          kind="ExternalInput").ap()
    w0l_d = nc.dram_tensor("w0l", [128, KP * 1024], F8L,
                           kind="ExternalInput").ap()
    w1_d = nc.dram_tensor("w1", [128, KP * 1024], F8,
                          kind="ExternalInput").ap()
    w1l_d = nc.dram_tensor("w1l", [128, KP * 1024], F8L,
                           kind="ExternalInput").ap()
    wpc_d = nc.dram_tensor("wpc", [128, 2 * 8 * 512], F32R,
                           kind="ExternalInput").ap()
    xpc_d = nc.dram_tensor("xpc", [128, 2 * 8 * BC], F32R,
                           kind="ExternalInput").ap()
    sc_d = nc.dram_tensor("scT", [128, 4], BF16, kind="ExternalInput").ap()
    lm_d = nc.dram_tensor("lmask", [BC, Mb], F32, kind="ExternalInput").ap()
    out_d = nc.dram_tensor("out", [BC, Mb], F32, kind="ExternalOutput").ap()

    with tile.TileContext(nc) as tc:
        with (
            tc.tile_pool(name="consts", bufs=1) as consts,
            tc.tile_pool(name="xr", bufs=opts["xr_bufs"]) as xpool,
            tc.tile_pool(name="acts", bufs=opts["c_bufs"]) as cpool,
            tc.tile_pool(name="ps", bufs=opts["ps_bufs"], space="PSUM") as pspool,
            tc.tile_pool(name="pssm", bufs=1, space="PSUM") as psmall,
            tc.tile_pool(name="epi", bufs=1) as epi,
            tc.tile_pool(name="dram", bufs=1, space="DRAM") as dpool,
        ):
            # ---- constants -------------------------------------------------
            wh_t = consts.tile([128, KD * 1024], F8, tag="wh")
            nc.sync.dma_start(wh_t[:], wh_d)
            w0_t = consts.tile([128, KP * 1024], F8, tag="w0")
            nc.sync.dma_start(w0_t[:], w0_d)
            w1_t = consts.tile([128, KP * 1024], F8, tag="w1")
            nc.sync.dma_start(w1_t[:], w1_d)
            whl_t = w0l_t = w1l_t = None
            if whh:
                whl_t = consts.tile([128, KD * 1024], F8L, tag="whl")
                nc.sync.dma_start(whl_t[:], whl_d)
            if wh_:
                w0l_t = consts.tile([128, KP * 1024], F8L, tag="w0l")
                nc.sync.dma_start(w0l_t[:], w0l_d)
                w1l_t = consts.tile([128, KP * 1024], F8L, tag="w1l")
                nc.sync.dma_start(w1l_t[:], w1l_d)
            wpc_t = consts.tile([128, 2 * 8 * 512], F32R, tag="wpc")
            nc.sync.dma_start(wpc_t[:], wpc_d)
            xpc_t = consts.tile([128, 2 * 8 * BC], F32R, tag="xpc")
            nc.sync.dma_start(xpc_t[:], xpc_d)
            sc_t = consts.tile([128, 4], BF16, tag="sc")
            nc.sync.dma_start(sc_t[:], sc_d)
            oh_t = consts.tile([BC, nblk * 512], F32R, tag="oh")
            nc.sync.dma_start(oh_t[:], oh_d)
            lm_t = consts.tile([BC, Mb], F32, tag="lm")
            nc.gpsimd.dma_start(lm_t[:], lm_d)

            # ---- per-batch bias rows: [32 b, 512 j] via transposed matmul --
            ps_bias = pspool.tile([BC, 512], F32, tag="ps", name="ps_bias")
            n_acc = 0
            for half in range(2):       # prep, child
                for dk in range(8):
                    nc.tensor.matmul(
                        ps_bias[:],
                        xpc_t[:, half * 8 * BC + dk * BC:
                              half * 8 * BC + (dk + 1) * BC],
                        wpc_t[:, half * 8 * 512 + dk * 512:
                              half * 8 * 512 + (dk + 1) * 512],
                        start=(n_acc == 0),
                        stop=(n_acc == 15),
                    )
                    n_acc += 1
            bias_r = consts.tile([BC, 512], F32R, tag="bias_r")
            nc.vector.tensor_copy(bias_r[:], ps_bias[:])

            # scores row accumulated across blocks, then bounced via DRAM
            exprow = epi.tile([1, N], F32, tag="exprow")
            srow_d = dpool.tile([1, N], F32, tag="srow")

            # ---- software-pipelined main loop -----------------------------
            def emit_s1(g):
                """stage 1 of block g -> c1 (and optional hi/lo) tiles."""
                xr = xpool.tile([128, KD * 1024], F8, tag="xr",
                                name=f"xr_{g}")
                hw = KD * 1024 // 2
                for h in range(2):
                    nc.sync.dma_start(
                        xr[:, h * hw: (h + 1) * hw],
                        xT_d[g, :, h * hw: (h + 1) * hw],
                    )
                xrl = None
                if xh:
                    xrl = xpool.tile([128, KD * 1024], F8L, tag="xrl",
                                     name=f"xrl_{g}")
                    for h in range(2):
                        nc.sync.dma_start(
                            xrl[:, h * hw: (h + 1) * hw],
                            xTl_d[g, :, h * hw: (h + 1) * hw],
                        )
                c1 = [cpool.tile([128, 1024], F8, tag="c1",
                                 name=f"c1_{g}_{t}") for t in range(2)]
                c1f = c1lo = None
                if c1h:
                    c1f = [cpool.tile([128, 1024], F32, tag="c1f",
                                      name=f"c1f_{g}_{t}") for t in range(2)]
                    c1lo = [cpool.tile([128, 1024], F8L, tag="c1l",
                                       name=f"c1l_{g}_{t}") for t in range(2)]
                for jt in range(4):
                    ps1 = pspool.tile([128, 512], F32, tag="ps",
                                      name=f"ps1_{g}_{jt}")
                    nc.tensor.matmul(
                        ps1[:],
                        bias_r[:, jt * 128: (jt + 1) * 128],
                        oh_t[:, g * 512: (g + 1) * 512],
                        start=True, stop=False,
                    )
                    mms = [(wh_t, xr)]
                    if whh:
                        mms.append((whl_t, xr))
                    if xh:
                        mms.append((wh_t, xrl))
                    for im, (wsrc, xsrc) in enumerate(mms):
                        for kt in range(KD):
                            nc.tensor.matmul(
                                ps1[:],
                                _dr3(wsrc[:, kt * 1024: (kt + 1) * 1024])
                                [:, :, jt * 128: (jt + 1) * 128],
                                _dr3(xsrc[:, kt * 1024: (kt + 1) * 1024]),
                                start=False,
                                stop=(im == len(mms) - 1 and kt == KD - 1),
                                perf_mode=DR,
                            )
                    dst = c1f if c1h else c1
                    nc.scalar.activation(
                        dst[jt // 2][:, (jt % 2) * 512: (jt % 2 + 1) * 512],
                        ps1[:], AF.Tanh,
                    )
                if c1h:
                    for t in range(2):
                        nc.gpsimd.tensor_copy(c1[t][:], c1f[t][:])
                        nc.vector.tensor_sub(c1lo[t][:], c1f[t][:], c1[t][:])
                return c1, c1lo

            def emit_s23(g, c1, c1lo):
                """stages 2/3 + scorer of block g."""
                c2 = [cpool.tile([128, 1024], F8, tag="c2",
                                 name=f"c2_{g}_{t}") for t in range(2)]
                c2f = c2lo = None
                if c2h:
                    c2f = [cpool.tile([128, 1024], F32, tag="c2f",
                                      name=f"c2f_{g}_{t}") for t in range(2)]
                    c2lo = [cpool.tile([128, 1024], F8L, tag="c2l",
                                       name=f"c2l_{g}_{t}") for t in range(2)]
                for qt in range(4):
                    ps2 = pspool.tile([128, 512], F32, tag="ps",
                                      name=f"ps2_{g}_{qt}")
                    mms = [(w0_t, c1)]
                    if c1h:
                        mms.append((w0_t, c1lo))
                    if wh_:
                        mms.append((w0l_t, c1))
                    for im, (wsrc, csrc) in enumerate(mms):
                        for kt in range(KP):
                            nc.tensor.matmul(
                                ps2[:],
                                _dr3(wsrc[:, kt * 1024: (kt + 1) * 1024])
                                [:, :, qt * 128: (qt + 1) * 128],
                                _dr3(csrc[kt][:]),
                                start=(im == 0 and kt == 0),
                                stop=(im == len(mms) - 1 and kt == KP - 1),
                                perf_mode=DR,
                            )
                    dst = c2f if c2h else c2
                    nc.scalar.activation(
                        dst[qt // 2][:, (qt % 2) * 512: (qt % 2 + 1) * 512],
                        ps2[:], AF.Tanh,
                    )
                if c2h:
                    for t in range(2):
                        nc.vector.tensor_copy(c2[t][:], c2f[t][:])
                        nc.gpsimd.tensor_sub(c2lo[t][:], c2f[t][:], c2[t][:])

                c3 = [cpool.tile([128, 1024], BF16, tag="c3",
                                 name=f"c3_{g}_{t}") for t in range(2)]
                for qt in range(4):
                    ps3 = pspool.tile([128, 512], F32, tag="ps",
                                      name=f"ps3_{g}_{qt}")
                    mms = [(w1_t, c2)]
                    if c2h:
                        mms.append((w1_t, c2lo))
                    if wh_:
                        mms.append((w1l_t, c2))
                    for im, (wsrc, csrc) in enumerate(mms):
                        for kt in range(KP):
                            nc.tensor.matmul(
                                ps3[:],
                                _dr3(wsrc[:, kt * 1024: (kt + 1) * 1024])
                                [:, :, qt * 128: (qt + 1) * 128],
                                _dr3(csrc[kt][:]),
                                start=(im == 0 and kt == 0),
                                stop=(im == len(mms) - 1 and kt == KP - 1),
                                perf_mode=DR,
                            )
                    nc.scalar.activation(
                        c3[qt // 2][:, (qt % 2) * 512: (qt % 2 + 1) * 512],
                        ps3[:], AF.Tanh,
                    )

                pss = psmall.tile([1, 512], F32, tag="pss", name=f"pss_{g}")
                for qk in range(4):
                    nc.tensor.matmul(
                        pss[:],
                        sc_t[:, qk: qk + 1],
                        c3[qk // 2][:, (qk % 2) * 512: (qk % 2 + 1) * 512],
                        start=(qk == 0), stop=(qk == 3),
                    )
                nc.vector.tensor_copy(
                    exprow[:, g * 512: (g + 1) * 512], pss[:]
                )

            for _rep in range(reps):
                prev = None
                for it in range(nblk + 1):
                    if it < nblk:
                        cur = emit_s1(it)
                    if prev is not None:
                        emit_s23(it - 1, *prev)
                    prev = cur if it < nblk else None

                # ---- epilogue: [32, Mb] reshape via DRAM, normalize -------
                nc.gpsimd.dma_start(srow_d[:], exprow[:])
                esc = epi.tile([BC, Mb], F32, tag="esc", name="esc")
                nc.gpsimd.dma_start(
                    esc[:], srow_d[:].rearrange("o (b r) -> (o b) r", b=BC)
                )
                zm = epi.tile([BC, Mb], F32, tag="zm", name="zm")
                nc.vector.tensor_add(zm[:], esc[:], lm_t[:])
                em = epi.tile([BC, Mb], F32, tag="em", name="em")
                sums = epi.tile([BC, 1], F32, tag="sums", name="sums")
                nc.scalar.activation(em[:], zm[:], AF.Exp, accum_out=sums[:])
                nc.vector.tensor_scalar_add(sums[:], sums[:], EPS)
                rec = epi.tile([BC, 1], F32, tag="rec", name="rec")
                nc.vector.reciprocal(rec[:], sums[:])
                outv = epi.tile([BC, Mb], F32, tag="outv", name="outv")
                nc.vector.tensor_scalar_mul(outv[:], em[:], rec[:])
                nc.sync.dma_start(out_d[:], outv[:])

    _split_waits(nc)
    return nc


# ---------------------------------------------------------------------------
def _dr_layout(w, kt, np_dt):
    """[K, M] -> [128, kt*1024] DoubleRow k-tile layout."""
    K, M = w.shape
    return (w.reshape(kt, 2, 128, M).transpose(2, 0, 1, 3)
            .reshape(128, kt * 2 * M).astype(np_dt))


def _host_prep(x, proj_head, proj_prep, proj_child, hidden_layers, scorer,
               mask, opts=None):
    opts = dict(OPTS, **(opts or {}))
    x = np.asarray(x, np.float32)
    mask = np.asarray(mask, bool)

    # global stripe width (uniform across cores for SPMD)
    cnts = mask[:, : S - 2].sum(axis=1).astype(np.int64)
    Mb = max(16, int(-(-cnts.max() // 16)) * 16)
    N = BC * Mb
    nblk = N // 512

    def hilo(w):
        hi = w.astype(NP_F8).astype(np.float32)
        return hi, (w - hi)

    wh = np.asarray(proj_head, np.float32)              # [D, P]
    wh_hi, wh_lo = hilo(wh)
    wh_l = _dr_layout(wh_hi, KD, NP_F8)
    whl_l = _dr_layout(wh_lo, KD, NP_F8L)
    hl = np.asarray(hidden_layers, np.float32)
    w_hi = [hilo(hl[i]) for i in range(2)]
    w_l = [_dr_layout(w_hi[i][0], KP, NP_F8) for i in range(2)]
    wl_l = [_dr_layout(w_hi[i][1], KP, NP_F8L) for i in range(2)]
    wp = np.asarray(proj_prep, np.float32)
    wc = np.asarray(proj_child, np.float32)
    wpc = np.concatenate(
        [w.reshape(8, 128, P).transpose(1, 0, 2).reshape(128, 8 * P)
         for w in (wp, wc)], axis=1
    ).astype(np.float32)
    scT = (np.asarray(scorer, np.float32).reshape(4, 128).T
           .astype(NP_BF16))                            # [128, 4]

    # one-hot batch-of-column rows (identical for every core)
    colb = np.arange(512)[None, None, :]
    gidx = np.arange(nblk)[None, :, None]
    bidx = np.arange(BC)[:, None, None]
    oh = ((512 * gidx + colb) // Mb == bidx).astype(np.float32)
    oh = oh.reshape(BC, nblk * 512)

    in_maps = []
    scatter = []
    for c in range(NCORES):
        xb = x[c * BC: (c + 1) * BC]                    # [32, 256, 1024]
        mb = mask[c * BC: (c + 1) * BC, : S - 2]
        Xc = np.zeros((BC, Mb, D), np.float32)
        lm = np.full((BC, Mb), NEG, np.float32)
        idxs = []
        for b in range(BC):
            idx = np.nonzero(mb[b])[0]
            Xc[b, : len(idx)] = xb[b, idx]
            lm[b, : len(idx)] = 0.0
            idxs.append(idx)
        scatter.append(idxs)
        Xf = Xc.reshape(N, D)
        Xhi = Xf.astype(NP_F8)

        def xlay(a, np_dt):
            return (np.ascontiguousarray(
                a.astype(np_dt).reshape(nblk, 512, KD, 2, 128)
                .transpose(0, 4, 2, 3, 1))
                .reshape(nblk, 128, KD * 1024))

        xT = xlay(Xf, NP_F8)
        if opts["x_hilo"]:
            xTl = xlay(Xf - Xhi.astype(np.float32), NP_F8L)
        else:
            xTl = np.zeros((nblk, 128, KD * 1024), NP_F8L)
        xpc = np.concatenate(
            [xb[:, S - 2 + half, :].T.reshape(8, 128, BC)
             .transpose(1, 0, 2).reshape(128, 8 * BC)
             for half in range(2)], axis=1
        ).astype(np.float32)
        in_maps.append(
            {
                "xT": xT, "xTl": xTl, "oh": oh,
                "wh": wh_l, "whl": whl_l,
                "w0": w_l[0], "w0l": wl_l[0], "w1": w_l[1], "w1l": wl_l[1],
                "wpc": wpc, "xpc": xpc, "scT": scT, "lmask": lm,
            }
        )
    return in_maps, scatter, Mb, nblk


_NC_CACHE = {}


def _get_nc(nblk, key="default"):
    ck = (nblk, key)
    if ck not in _NC_CACHE:
        _NC_CACHE[ck] = _build(nblk)
    return _NC_CACHE[ck]


def kernel(x, proj_head, proj_prep, proj_child, hidden_layers, scorer, mask):
    in_maps, scatter, Mb, nblk = _host_prep(
        x, proj_head, proj_prep, proj_child, hidden_layers, scorer, mask
    )
    nc = _get_nc(nblk)
    res = bass_utils.run_bass_kernel_spmd(
        nc, in_maps, core_ids=list(range(NCORES))
    )
    out = np.zeros((B, S - 2), np.float32)
    for c in range(NCORES):
        rows = res.results[c]["out"]                    # [BC, Mb]
        for b in range(BC):
            idx = scatter[c][b]
            out[c * BC + b, idx] = rows[b, : len(idx)]
    return out


if __name__ == "__main__":
    rng = np.random.default_rng(0)
    x = rng.standard_normal((B, S, D)).astype(np.float32)
    u = lambda shp: rng.uniform(-0.05, 0.05, shp).astype(np.float32)
    inputs = dict(
        x=x, proj_head=u((D, P)), proj_prep=u((D, P)), proj_child=u((D, P)),
        hidden_layers=u((2, P, P)), scorer=u((P,)),
        mask=rng.integers(0, 2, (B, S)).astype(bool),
    )
    out = kernel(**inputs)
    print("kernel out", out.shape, out.dtype, out[:2, :4])
